# revision 1
# baseline (speedup 1.0000x reference)
"""Masked dot-product attention on 8 Trainium2 NeuronCores (Bass/Tile).

Problem: queries/keys/values [32, 1024, 128] f32, valid_lens [32] i32.
  out = softmax(mask(Q K^T / sqrt(128))) V        (key-padding prefix mask)

Strategy (batch-parallel, 4 batches per core, one SPMD program):
  * Host pre-transposes Q and K per batch to [D=128, 1024] so the
    contraction dim D sits on SBUF partitions; no on-device transposes.
  * Scores are computed transposed: S^T[k, q] = (K^T chunk).T @ Q^T with k
    in chunks of 128 partitions.
  * The prefix key mask is per-PARTITION in this layout, so it folds into
    the exp for free: ACT computes exp(S^T * 1/sqrt(D) + bias) with
    bias[k] in {0, -1e6}; masked rows become exactly 0.
  * out^T[v, q] += V_chunk-as-lhsT @ expS^T accumulates in PSUM across
    k chunks (V is loaded chunk-major, no transpose needed).
  * denominator[q]: expS^T chunks are summed in PAIRS on DVE, then one
    ones-column matmul per pair accumulates in PSUM -- this halves the
    PE cost of the reduction, which otherwise paces the whole kernel.
  * out^T and sums are DMA'd back; the host divides and transposes
    while gathering (0.003% of the FLOPs).
  * float32r everywhere on the PE: 1 cycle/row instead of fp32's 4.

Static masked-chunk skipping: batch b only needs ceil(valid_lens[b]/128)
key chunks; the rest contribute exactly 0. Batches are assigned to the 4
per-core slots by descending need (sorted, slot-major), so slot j's
compile-time chunk count is max over its 8 batches. The SPMD program is
specialized to that profile at kernel build time.

The chunk loop is software-pipelined: chunk c+1's score matmuls are
emitted before chunk c's AV/sums matmuls so the PE produces the next
exp's input first and ACT never starves.
"""

import math

import numpy as np

import concourse.bacc as bacc
import concourse.bass as bass
import concourse.mybir as mybir
import concourse.tile as tile
from concourse.bass_utils import run_bass_kernel_spmd

B, Q, K, D = 32, 1024, 1024, 128
N_CORES = 8
BPC = B // N_CORES  # batches per core
PART = 128          # partition size / key chunk size
NCHUNK = K // PART
MASK_BIAS = -1.0e6
INV_SQRT_D = 1.0 / math.sqrt(D)
F32 = mybir.dt.float32
F32R = mybir.dt.float32r

_NC_CACHE: dict = {}


def build_nc(profile: tuple) -> bass.Bass:
    """Build the SPMD Bass program for a per-slot chunk-count profile."""
    nc = bacc.Bacc()
    qt = nc.declare_dram_parameter("qt", [BPC, PART, Q], F32R, isOutput=False)
    kt = nc.declare_dram_parameter("kt", [BPC, PART, K], F32R, isOutput=False)
    vp = nc.declare_dram_parameter("vp", [BPC, PART, K], F32R, isOutput=False)
    mb = nc.declare_dram_parameter("mb", [PART, BPC * NCHUNK], F32, isOutput=False)
    cst = nc.declare_dram_parameter("cst", [PART, PART], F32R, isOutput=False)
    out = nc.declare_dram_parameter("out", [BPC, PART, Q], F32, isOutput=True)
    sums_out = nc.declare_dram_parameter("sums", [BPC, 1, Q], F32, isOutput=True)

    with tile.TileContext(nc) as tc:
        with (
            tc.tile_pool(name="io", bufs=2) as io,
            tc.tile_pool(name="probs", bufs=2) as probs,
            tc.tile_pool(name="consts", bufs=1) as consts,
            tc.tile_pool(name="ps_s", bufs=2, space="PSUM") as ps_s,
            tc.tile_pool(name="ps_acc", bufs=1, space="PSUM") as ps_acc,
        ):
            # Startup-ordered loads on the FIFO SP HWDGE ring: batch 0's
            # score operands first, small consts next, V loads trail (AV
            # consumes V only after the corresponding exp).
            ins_sb = []
            for b in range(BPC):
                cap = profile[b]
                kcols = cap * PART
                qt_sb = io.tile([PART, Q], F32R, tag="qt", name=f"qt{b}")
                kt_sb = io.tile([PART, kcols], F32R, tag="kt", name=f"kt{b}")
                vp_sb = io.tile([PART, kcols], F32R, tag="vp", name=f"vp{b}")
                ins_sb.append((qt_sb, kt_sb, vp_sb))
                nc.sync.dma_start(out=kt_sb, in_=kt[b][:, :kcols])
                nc.sync.dma_start(out=qt_sb, in_=qt[b])
                if b == 0:
                    mb_sb = consts.tile([PART, BPC * NCHUNK], F32)
                    nc.sync.dma_start(out=mb_sb, in_=mb[:, :])
                    cst_sb = consts.tile([PART, PART], F32R)
                    nc.sync.dma_start(out=cst_sb, in_=cst[:, :])
                    ones_col = cst_sb[:, 0:1]
            for b in range(BPC):
                nc.sync.dma_start(
                    out=ins_sb[b][2], in_=vp[b][:, :profile[b] * PART]
                )

            # Flat chunk stream across batches with 2-deep score lookahead:
            # the in-order PE queue must see the next chunks' score matmuls
            # BEFORE a batch-boundary AV matmul that may stall on the PSUM
            # accumulator release.
            stream = [(b, c) for b in range(BPC) for c in range(profile[b])]

            def s_mms(b, c):
                qt_sb, kt_sb, _ = ins_sb[b]
                s_ps = ps_s.tile([PART, Q], F32, tag="s", name=f"s_b{b}c{c}")
                kw = kt_sb[:, c * PART:(c + 1) * PART]
                for h in range(2):
                    nc.tensor.matmul(
                        s_ps[:, h * 512:(h + 1) * 512],
                        kw,
                        qt_sb[:, h * 512:(h + 1) * 512],
                        start=True,
                        stop=True,
                    )
                return s_ps

            s_tiles = {}
            for j in range(min(2, len(stream))):
                s_tiles[stream[j]] = s_mms(*stream[j])
            acc = {}
            prev_p = {}
            pend_sums = []
            for i, (b, c) in enumerate(stream):
                cap = profile[b]
                if c == 0:
                    out_ps = ps_acc.tile(
                        [PART, Q], F32, tag="out", name=f"out_b{b}"
                    )
                    sums_ps = ps_acc.tile(
                        [1, Q], F32, tag="sums", name=f"sums_b{b}"
                    )
                    acc[b] = (out_ps, sums_ps)
                out_ps, sums_ps = acc[b]
                p_sb = probs.tile([PART, Q], F32R, tag="p", bufs=32, name=f"p_{i}")
                nc.scalar.activation(
                    p_sb,
                    s_tiles.pop((b, c)),
                    mybir.ActivationFunctionType.Exp,
                    bias=mb_sb[:, b * NCHUNK + c:b * NCHUNK + c + 1],
                    scale=INV_SQRT_D,
                )
                if i + 2 < len(stream):
                    s_tiles[stream[i + 2]] = s_mms(*stream[i + 2])
                # Deferred pair-sums matmuls: emitted one iteration late so
                # the DVE add has a full chunk-time to finish and never
                # stalls the in-order PE queue ahead of the score matmuls.
                for ps_t, rhs_t, st, sp in pend_sums:
                    for h in range(2):
                        nc.tensor.matmul(
                            ps_t[:, h * 512:(h + 1) * 512],
                            ones_col[:, :],
                            rhs_t[:, h * 512:(h + 1) * 512],
                            start=st,
                            stop=sp,
                        )
                pend_sums.clear()
                vw = ins_sb[b][2][:, c * PART:(c + 1) * PART]
                first, last = c == 0, c == cap - 1
                for h in range(2):
                    nc.tensor.matmul(
                        out_ps[:, h * 512:(h + 1) * 512],
                        vw,
                        p_sb[:, h * 512:(h + 1) * 512],
                        start=first,
                        stop=last,
                    )
                if cap == 1 or (c == cap - 1 and c % 2 == 0):
                    pend_sums.append((sums_ps, p_sb, c == 0, True))
                elif c % 2 == 0:
                    prev_p[b] = p_sb
                else:
                    pair_sb = probs.tile(
                        [PART, Q], F32R, tag="p", bufs=32, name=f"pair_{i}"
                    )
                    nc.vector.tensor_add(pair_sb, prev_p.pop(b), p_sb)
                    pend_sums.append((sums_ps, pair_sb, c == 1, c >= cap - 2))
                if last:
                    # Flush this batch's remaining deferred sums matmuls
                    # before the epilogue reads sums_ps.
                    for ps_t, rhs_t, st, sp in pend_sums:
                        for h in range(2):
                            nc.tensor.matmul(
                                ps_t[:, h * 512:(h + 1) * 512],
                                ones_col[:, :],
                                rhs_t[:, h * 512:(h + 1) * 512],
                                start=st,
                                stop=sp,
                            )
                    pend_sums.clear()
                    # Epilogue: PSUM -> SBUF split across ACT and DVE so the
                    # accumulator releases quickly, then DMA; host divides.
                    # ACT copies PSUM ~2.4x faster than DVE; split so both
                    # halves finish together (~0.85us) and out_ps releases
                    # sooner for the next batch's accumulation.
                    outn = io.tile([PART, Q], F32, tag="outn", bufs=4, name=f"outn{b}")
                    nc.scalar.copy(outn[:, 0:720], out_ps[:, 0:720])
                    nc.sync.dma_start(out=out[b][:, 0:720], in_=outn[:, 0:720])
                    nc.vector.tensor_copy(outn[:, 720:1024], out_ps[:, 720:1024])
                    nc.sync.dma_start(
                        out=out[b][:, 720:1024], in_=outn[:, 720:1024]
                    )
                    sums_sb = probs.tile(
                        [1, Q], F32, tag="sums_sb", bufs=4, name=f"sums_sb{b}"
                    )
                    nc.vector.tensor_copy(sums_sb, sums_ps)
                    nc.sync.dma_start(out=sums_out[b], in_=sums_sb)

    nc.compile()
    return nc


def plan(valid_lens: np.ndarray):
    """Assign batches to (core, slot) and derive the chunk-count profile.

    Sorting by descending need and slicing slot-major minimizes the sum of
    per-slot maxima, which is the per-core static work.
    """
    need = np.minimum((valid_lens.astype(np.int64) + PART - 1) // PART, NCHUNK)
    need = np.maximum(need, 1)
    order = np.argsort(-need, kind="stable")
    perm = order.reshape(BPC, N_CORES)  # perm[slot, core] = batch index
    # Process the smallest slot first: its input DMAs are the ones compute
    # must wait for at startup; the bigger slots' loads overlap compute.
    rot = np.argsort([int(need[perm[s]].max()) for s in range(BPC)], kind="stable")
    rot = np.concatenate([rot[:1], rot[1:][::-1]])  # smallest, then descending
    perm = perm[rot]
    profile = tuple(int(need[perm[s]].max()) for s in range(BPC))
    return perm, profile


def kernel(queries, keys, values, valid_lens):
    q = np.ascontiguousarray(np.asarray(queries, dtype=np.float32))
    k = np.ascontiguousarray(np.asarray(keys, dtype=np.float32))
    v = np.ascontiguousarray(np.asarray(values, dtype=np.float32))
    lens = np.asarray(valid_lens).astype(np.int64).reshape(B)

    perm, profile = plan(lens)

    if profile not in _NC_CACHE:
        _NC_CACHE[profile] = build_nc(profile)
    nc = _NC_CACHE[profile]

    # Vectorized host layout prep: obi[core, slot] = batch index.
    obi = perm.T  # [N_CORES, BPC]
    qt_all = q[obi].transpose(0, 1, 3, 2)  # [8,4,128,1024]
    kt_all = k[obi].transpose(0, 1, 3, 2)
    # v chunk-major: vp[p, c*128 + d] = v[c*128 + p, d]
    vp_all = np.ascontiguousarray(
        v[obi]
        .reshape(N_CORES, BPC, NCHUNK, PART, D)
        .transpose(0, 1, 3, 2, 4)
        .reshape(N_CORES, BPC, PART, K)
    )
    # bias[p, slot*8 + c] = 0 if (c*128+p) < L else -1e6
    valid = np.arange(K)[None, None, :] < lens[obi][:, :, None]  # [8,4,1024]
    mb_all = np.where(
        valid.reshape(N_CORES, BPC, NCHUNK, PART).transpose(0, 2, 3, 1), 0.0, MASK_BIAS
    ).astype(np.float32)  # [8, NCHUNK, PART, BPC] -> need [8, PART, BPC*NCHUNK]
    mb_all = np.ascontiguousarray(
        mb_all.transpose(0, 2, 3, 1).reshape(N_CORES, PART, BPC * NCHUNK)
    )
    ones = np.ones((PART, PART), np.float32)

    qt_all = np.ascontiguousarray(qt_all)
    kt_all = np.ascontiguousarray(kt_all)
    in_maps = [
        {
            "qt": qt_all[core],
            "kt": kt_all[core],
            "vp": vp_all[core],
            "mb": mb_all[core],
            "cst": ones,
        }
        for core in range(N_CORES)
    ]

    res = run_bass_kernel_spmd(nc, in_maps, list(range(N_CORES)))

    out = np.empty((B, Q, D), np.float32)
    for core in range(N_CORES):
        core_out = res.results[core]["out"]    # [BPC, 128(v), 1024(q)]
        core_sums = res.results[core]["sums"]  # [BPC, 1, 1024(q)]
        for slot in range(BPC):
            bidx = int(perm[slot, core])
            out[bidx] = (core_out[slot] / core_sums[slot]).T
    return out



# revision 10
# speedup vs baseline: 1.3058x; 1.3058x over previous
"""Masked dot-product attention on 8 Trainium2 NeuronCores (Bass/Tile).

Problem: queries/keys/values [32, 1024, 128] f32, valid_lens [32] i32.
  out = softmax(mask(Q K^T / sqrt(128))) V        (key-padding prefix mask)

v2 strategy — piece-parallel, ACT-bound pipeline:
  * Attention numerator (sum_k p_k v_k) and denominator (sum_k p_k) are
    ADDITIVE over key chunks, so a batch's key range can be cut into
    pieces processed on different slots/cores; the host sums partial
    numerators/denominators, divides, and transposes.
  * The planner cuts the 32 batches' chunk-needs (sum = ceil-per-128 of
    valid_lens) into pieces that EXACTLY fill an SPMD-identical slot
    profile across the 8 cores — per-core work drops to
    ceil(total_chunks/8) with zero padding when an exact packing exists
    (for the fixed harness input: 17 chunks/core vs 20 for slot-max).
  * All matmul operands are bf16 (same PE rate as f32r, half the DMA and
    SBUF), PSUM accumulation f32. exp runs on ACT (f32 PSUM in -> bf16
    out) — ACT is the bottleneck engine, so it does NOTHING else.
  * Scores are computed transposed: S^T[k, q] = K_chunk^T-as-lhsT @ Q^T;
    the prefix key mask is per-partition, folded into exp via ACT bias.
  * Per piece: AV accumulates V_chunk-as-lhsT @ p in PSUM across chunks;
    the denominator is a DVE bf16 add-tree over the piece's p tiles plus
    ONE ones-column matmul, keeping PE cost ~1/2 matmul per chunk.
  * Epilogue copies run on GpSimd (idle engine), out is DMA'd as bf16,
    sums as f32. Input DMAs are per-slot packed segments (kt|vp|qt), one
    trigger each, smallest slot first so the PE starts within ~1us.
  * PE program order: scores for chunk i+2 are emitted BEFORE the
    (deferred by one iteration) AV of chunk i-1 and any slot epilogue,
    so the in-order PE queue always produces ACT's next input first.
"""

import math

import numpy as np
import ml_dtypes

import concourse.bacc as bacc
import concourse.bass as bass
import concourse.mybir as mybir
import concourse.tile as tile
from concourse.bass_utils import run_bass_kernel_spmd

B, Q, K, D = 32, 1024, 1024, 128
N_CORES = 8
PART = 128
NCHUNK = K // PART
MASK_BIAS = -1.0e6
INV_SQRT_D = 1.0 / math.sqrt(D)
F32 = mybir.dt.float32
BF16 = mybir.dt.bfloat16
NPBF16 = ml_dtypes.bfloat16

_NC_CACHE: dict = {}
_PLAN_CACHE: dict = {}


# ---------------------------------------------------------------- planner
def _decompose(caps, sizes, counts):
    """Cut caps into parts drawn from `sizes` with exactly counts[s] parts
    of size s overall. Returns list of part-lists per cap, or None."""
    order = sorted(range(len(caps)), key=lambda i: -caps[i])
    sizes = sorted(sizes, reverse=True)
    comp_cache = {}

    def comps(c):
        if c in comp_cache:
            return comp_cache[c]
        out = []

        def rec(c, maxs, cur):
            if c == 0:
                out.append(tuple(cur))
                return
            for s in sizes:
                if s > maxs or s > c:
                    continue
                cur.append(s)
                rec(c - s, s, cur)
                cur.pop()

        rec(c, max(sizes), [])
        comp_cache[c] = out
        return out

    res = [None] * len(caps)
    cnt = dict(counts)
    nodes = [0]

    def dfs(i):
        nodes[0] += 1
        if nodes[0] > 120000:
            return False
        if i == len(order):
            return all(v == 0 for v in cnt.values())
        b = order[i]
        for comp in comps(caps[b]):
            ok = True
            for s in comp:
                cnt[s] -= 1
                if cnt[s] < 0:
                    ok = False
            if ok and dfs(i + 1):
                res[b] = list(comp)
                return True
            for s in comp:
                cnt[s] += 1
        return False

    return res if dfs(0) else None


def _partitions(total, max_part, max_count):
    results = []

    def rec(rem, maxp, cur):
        if rem == 0:
            results.append(tuple(cur))
            return
        if len(cur) >= max_count:
            return
        for p in range(min(maxp, rem), 0, -1):
            cur.append(p)
            rec(rem - p, p, cur)
            cur.pop()

    rec(total, max_part, [])
    results.sort(key=lambda t: (len(t), [-x for x in t]))
    return results


def plan(lens):
    """-> (profile, assign): profile = per-core slot caps in processing
    order (ascending); assign[core][slot] = (batch, chunk_start, n) or None
    (padded slot, fully masked)."""
    lens = np.asarray(lens).astype(np.int64)
    nb = len(lens)
    caps = [int(c) for c in
            np.maximum(np.minimum((lens + PART - 1) // PART, NCHUNK), 1)]
    total = sum(caps)

    found = None
    base_T = -(-total // N_CORES)
    for T in range(base_T, base_T + 2):
        for prof in _partitions(T, min(NCHUNK, T), 8):
            sizes = sorted(set(prof), reverse=True)
            if min(caps) < min(sizes):
                continue
            counts = {s: N_CORES * prof.count(s) for s in sizes}
            dec = _decompose(caps, sizes, counts)
            if dec is not None:
                found = (prof, dec)
                break
        if found:
            break

    if found is None:
        # fallback: classic slot-max scheme (always feasible, some padding)
        bpc = -(-nb // N_CORES)
        order = np.argsort([-c for c in caps], kind="stable")
        prof = tuple(max(caps[b] for b in order[s * N_CORES:(s + 1) * N_CORES])
                     for s in range(bpc))
        dec = [[caps[b]] for b in range(nb)]
        found = (prof, dec)

    prof, dec = found
    pieces_by_size = {}
    for b in range(nb):
        start = 0
        for part in sorted(dec[b], reverse=True):
            pieces_by_size.setdefault(part, []).append((b, start, part))
            start += part

    order_prof = tuple(sorted(prof))
    assign = [[] for _ in range(N_CORES)]
    idx = {s: 0 for s in pieces_by_size}
    for cap in order_prof:
        for core in range(N_CORES):
            lst = pieces_by_size.get(cap)
            if lst is not None and idx.get(cap, 0) < len(lst):
                assign[core].append(lst[idx[cap]])
                idx[cap] += 1
            else:
                assign[core].append(None)
    return order_prof, assign


# ----------------------------------------------------------- bass program
def build_nc(profile: tuple) -> bass.Bass:
    nc = bacc.Bacc()
    S = len(profile)
    tot = sum(profile)

    # per-slot packed input segment: [ kt (cap*128) | vp (cap*128) | qt (1024) ]
    ins_d = [
        nc.declare_dram_parameter(
            f"ins{s}", [PART, 2 * profile[s] * PART + Q], BF16, isOutput=False
        )
        for s in range(S)
    ]
    mb_d = nc.declare_dram_parameter("mb", [PART, tot], F32, isOutput=False)
    out_d = nc.declare_dram_parameter("out", [S, PART, Q], BF16, isOutput=True)
    # per-piece partition-wise sums of p (host reduces over partitions)
    gsum_d = nc.declare_dram_parameter("gsum", [S, PART, Q], BF16, isOutput=True)

    stream = [(s, c) for s, cap in enumerate(profile) for c in range(cap)]
    N = len(stream)
    pos_of = {}
    off = 0
    for s, cap in enumerate(profile):
        for c in range(cap):
            pos_of[(s, c)] = off + c
        off += cap
    last_of_slot = {s: sum(profile[:s + 1]) - 1 for s in range(S)}

    with tile.TileContext(nc) as tc:
        with (
            tc.tile_pool(name="ins", bufs=1) as insp,
            tc.tile_pool(name="consts", bufs=1) as consts,
            tc.tile_pool(name="probs", bufs=8) as probs,
            tc.tile_pool(name="accs", bufs=6) as accsp,
            tc.tile_pool(name="outsb", bufs=2) as outsbp,
            tc.tile_pool(name="ps_s", bufs=2, space="PSUM") as ps_s,
            tc.tile_pool(name="ps_out", bufs=2, space="PSUM") as ps_out,
        ):
            # input DMAs: smallest slot first; mask + per-slot segments
            mb_sb = consts.tile([PART, tot], F32, name="mb")
            nc.sync.dma_start(out=mb_sb, in_=mb_d[:, :])
            ins_sb = []
            for s in range(S):
                t = insp.tile(
                    [PART, 2 * profile[s] * PART + Q], BF16,
                    tag=f"ins{s}", name=f"ins{s}",
                )
                ins_sb.append(t)
                nc.sync.dma_start(out=t, in_=ins_d[s][:, :])

            def kt_ap(s, c):
                return ins_sb[s][:, c * PART:(c + 1) * PART]

            def vp_ap(s, c):
                cap = profile[s]
                return ins_sb[s][:, (cap + c) * PART:(cap + c + 1) * PART]

            def qt_ap(s, lo, hi):
                cap = profile[s]
                return ins_sb[s][:, 2 * cap * PART + lo:2 * cap * PART + hi]

            def s_mms(i):
                s, c = stream[i]
                sp = ps_s.tile([PART, Q], F32, tag="s", name=f"s{i}")
                kw = kt_ap(s, c)
                for h in range(2):
                    nc.tensor.matmul(
                        sp[:, h * 512:(h + 1) * 512],
                        kw,
                        qt_ap(s, h * 512, (h + 1) * 512),
                        start=True,
                        stop=True,
                    )
                return sp

            s_tiles = {}
            for j in range(min(2, N)):
                s_tiles[j] = s_mms(j)

            p_tiles = {}          # stream index -> p tile (bf16)
            acc_state = {}        # slot -> list of pending partial tiles
            out_ps_of = {}        # slot -> PSUM accumulator
            finish1 = []          # slots: emit AV-done epilogue copy
            finish2 = []          # slots: emit sums matmul + sums copy

            def emit_av(j):
                s, c = stream[j]
                cap = profile[s]
                if c == 0:
                    out_ps_of[s] = ps_out.tile(
                        [PART, Q], F32, tag="out", name=f"out_s{s}"
                    )
                op = out_ps_of[s]
                vw = vp_ap(s, c)
                pj = p_tiles[j]
                for h in range(2):
                    nc.tensor.matmul(
                        op[:, h * 512:(h + 1) * 512],
                        vw,
                        pj[:, h * 512:(h + 1) * 512],
                        start=(c == 0),
                        stop=(c == cap - 1),
                    )

            def emit_adds(j):
                # DVE bf16 add-tree over the slot's p tiles; leaves the
                # final reduced tile in acc_state[s][0] when slot complete.
                s, c = stream[j]
                cap = profile[s]
                st = acc_state.setdefault(s, [])
                st.append((p_tiles[j], 1))
                # merge equal-weight neighbors (binary counter) --> log tree
                while len(st) >= 2 and (
                    st[-1][1] == st[-2][1] or c == cap - 1
                ):
                    (a, na), (b_, nb) = st[-2], st[-1]
                    t = accsp.tile(
                        [PART, Q], BF16, tag="acc", name=f"acc{j}_{len(st)}"
                    )
                    nc.vector.tensor_add(t, a, b_)
                    st[-2:] = [(t, na + nb)]

            def emit_finish1(s):
                # out accumulator -> SBUF bf16 on DVE (only ACT/DVE read
                # PSUM), then DMA triggered from GpSimd
                osb = outsbp.tile([PART, Q], BF16, tag="osb", name=f"osb{s}")
                nc.vector.tensor_copy(osb, out_ps_of[s])
                nc.gpsimd.dma_start(out=out_d[s], in_=osb)

            def emit_finish2(s):
                # denominator: DMA the piece's partition-wise p-sum tile;
                # the host reduces over the 128 key partitions
                cap = profile[s]
                rhs = acc_state[s][0][0] if cap > 1 else p_tiles[last_of_slot[s]]
                nc.gpsimd.dma_start(out=gsum_d[s], in_=rhs)

            for i in range(N + 2):
                if i < N:
                    s, c = stream[i]
                    p = probs.tile([PART, Q], BF16, tag="p", name=f"p{i}")
                    nc.scalar.activation(
                        p,
                        s_tiles.pop(i),
                        mybir.ActivationFunctionType.Exp,
                        bias=mb_sb[:, pos_of[(s, c)]:pos_of[(s, c)] + 1],
                        scale=INV_SQRT_D,
                    )
                    p_tiles[i] = p
                    if i + 2 < N:
                        s_tiles[i + 2] = s_mms(i + 2)
                # deferred slot finishes (in order: frees PSUM earliest)
                for s_ in finish1:
                    emit_finish1(s_)
                finish1.clear()
                # deferred AV of previous chunk
                if 0 <= i - 1 < N:
                    emit_av(i - 1)
                    if profile[stream[i - 1][0]] > 1:
                        emit_adds(i - 1)
                    sl, cl = stream[i - 1]
                    if cl == profile[sl] - 1:
                        finish1.append(sl)
                for s_ in finish2:
                    emit_finish2(s_)
                finish2.clear()
                if 0 <= i - 1 < N:
                    sl, cl = stream[i - 1]
                    if cl == profile[sl] - 1:
                        finish2.append(sl)
            for s_ in finish1:
                emit_finish1(s_)
            for s_ in finish2:
                emit_finish2(s_)

    nc.compile()
    return nc


# ------------------------------------------------------------------ host
def _prep_core_inputs(core, profile, assign, qT, kT, v, lens):
    """Build the per-slot packed input segments + mask for one core."""
    S = len(profile)
    tot = sum(profile)
    ins = []
    mb = np.empty((PART, tot), np.float32)
    pos = 0
    for s in range(S):
        cap = profile[s]
        seg = np.zeros((PART, 2 * cap * PART + Q), NPBF16)
        pc = assign[core][s]
        if pc is not None:
            b, st, n = pc
            k0, k1 = st * PART, (st + n) * PART
            seg[:, 0:n * PART] = kT[b][:, k0:k1]
            seg[:, cap * PART:(cap + n) * PART] = (
                v[b][k0:k1]
                .reshape(n, PART, D)
                .transpose(1, 0, 2)
                .reshape(PART, n * PART)
            )
            seg[:, 2 * cap * PART:] = qT[b]
            kidx = np.arange(st * PART, (st + cap) * PART).reshape(cap, PART).T
            mb[:, pos:pos + cap] = np.where(kidx < lens[b], 0.0, MASK_BIAS)
        else:
            mb[:, pos:pos + cap] = MASK_BIAS
        ins.append(seg)
        pos += cap
    m = {f"ins{s}": ins[s] for s in range(S)}
    m["mb"] = np.ascontiguousarray(mb)
    return m


def kernel(queries, keys, values, valid_lens):
    q = np.asarray(queries, dtype=np.float32)
    k = np.asarray(keys, dtype=np.float32)
    v = np.asarray(values, dtype=np.float32)
    lens = np.asarray(valid_lens).astype(np.int64).reshape(B)

    key = tuple(int(x) for x in lens)
    if key not in _PLAN_CACHE:
        _PLAN_CACHE[key] = plan(lens)
    profile, assign = _PLAN_CACHE[key]

    if profile not in _NC_CACHE:
        _NC_CACHE[profile] = build_nc(profile)
    nc = _NC_CACHE[profile]

    qT = np.ascontiguousarray(q.transpose(0, 2, 1)).astype(NPBF16)  # [B,128,1024]
    kT = np.ascontiguousarray(k.transpose(0, 2, 1)).astype(NPBF16)
    vb = v.astype(NPBF16)

    in_maps = [
        _prep_core_inputs(core, profile, assign, qT, kT, vb, lens)
        for core in range(N_CORES)
    ]

    res = run_bass_kernel_spmd(nc, in_maps, list(range(N_CORES)))

    num = np.zeros((B, PART, Q), np.float32)   # [v, q] per batch
    den = np.zeros((B, 1, Q), np.float32)
    for core in range(N_CORES):
        co = res.results[core]["out"]    # [S, 128, 1024] bf16
        cs = res.results[core]["gsum"]   # [S, 128, 1024] bf16
        for s, pc in enumerate(assign[core]):
            if pc is None:
                continue
            b = pc[0]
            num[b] += co[s].astype(np.float32)
            den[b] += cs[s].astype(np.float32).sum(axis=0, keepdims=True)
    return np.ascontiguousarray((num / den).transpose(0, 2, 1))
